# revision 52
# baseline (speedup 1.0000x reference)
"""AttentionWithRope Trainium2 Bass kernel (v3, fp16 datapath, pipelined).

Sharding: 8 cores = 2 batches x 4 head-groups (4 heads / 256 features).
Per core: fp16 projections (biases folded in via the x ones-row), RoPE via
stream_shuffle (head features host-permuted so each rotate pair sits +-16
partitions apart inside a 32-partition shuffle window), causal attention
with the diagonal mask applied on the PE (accumulating (-60000*I)^T @
tril01 so exp underflows to exactly 0), transposed output projection.
Host sums the 4 partial outT per batch and adds wo_b.

Pipelining: one unified [128,1024] PSUM pool (4 bufs = all 8 banks) lets
the PE run ahead; z matmuls are delayed by one head so exp(h-1) hides
under scores(h); projection/output-projection groups for adjacent chunks
are emitted into the seams between attention heads so the PE never
starves while ACT drains exp (and stays out of the slow p-state). The
softmax normalization avoids the 3.3us DVE reciprocal on the critical
path: z psum is evacuated to SBUF (f32r), the sums row is broadcast
across 64 partitions by a tiny fp32r PE matmul (ones^T @ sums), and a
fast approximate reciprocal + multiply produce normalized fp16 zn. The
causal mask is applied on the PE by accumulating (-60000*I)^T @ tril01
into the diagonal score blocks (exp underflows to exactly 0). Seam
items are ordered q/k-projections first (they gate the next chunk's
scores), then out-projection tiles, then v; at j>=1 they are emitted at
score-pair granularity. x/weights/out use packed contiguous-per-partition DRAM layouts
(one descriptor-cheap DMA each), issued across both the SP and ACT
descriptor generators.
"""

import numpy as np
from contextlib import ExitStack

DIM, HEADS, HD = 1024, 16, 64
B, S = 2, 2048
NC = 8
HPC = 4          # heads per core
F = HPC * HD     # 256 features per core
ROPE_BASE = 10000.0


def _head_perm():
    # within-head feature order: [x1[0:16], x2[0:16], x1[16:32], x2[16:32]]
    return np.concatenate([np.arange(0, 16), np.arange(32, 48),
                           np.arange(16, 32), np.arange(48, 64)])


def _rope_tables():
    """CT/SST [128, S] fp32 in the permuted layout (2 heads per tile)."""
    theta = ROPE_BASE ** (-np.arange(0, HD, 2, dtype=np.float32) / HD)  # [32]
    pos = np.arange(S, dtype=np.float32)
    ang = pos[:, None] * theta[None, :]            # [S, 32]
    cos, sin = np.cos(ang).T, np.sin(ang).T        # [32, S]
    cA, cB = cos[0:16], cos[16:32]
    sA, sB = sin[0:16], sin[16:32]
    ct_head = np.concatenate([cA, cA, cB, cB], 0)              # [64, S]
    sst_head = np.concatenate([-sA, sA, -sB, sB], 0)           # [64, S]
    CT = np.concatenate([ct_head, ct_head], 0).astype(np.float32)    # [128, S]
    SST = np.concatenate([sst_head, sst_head], 0).astype(np.float32)
    return CT, SST


_SHUF_MASK = list(range(16, 32)) + list(range(0, 16))


def _build_program(debug=False):
    import concourse.bass as bass
    import concourse.mybir as mybir
    import concourse.tile as tile
    from concourse import bacc

    fp32 = mybir.dt.float32
    f32r = mybir.dt.float32r
    f16 = mybir.dt.float16
    AF = mybir.ActivationFunctionType

    nc = bacc.Bacc("TRN2", target_bir_lowering=False, num_devices=NC)

    from bass_rust import add_dep_helper as _adh
    _prev_mm = [None]

    def MM(*args, **kw):
        bi = nc.tensor.matmul(*args, **kw)
        if _prev_mm[0] is not None:
            _adh(bi.ins, _prev_mm[0].ins, sync=False, reason="pe-order")
        _prev_mm[0] = bi
        return bi

    # ---- DRAM I/O ----
    # xT packed as [4 chunks][128, 9*512]; weights packed partition-major
    xT_d = nc.dram_tensor("xT", [4, 128, 9 * 512], f16, kind="ExternalInput").ap()
    wqT_d = nc.dram_tensor("wqT", [128, 9 * F], f16, kind="ExternalInput").ap()
    wkT_d = nc.dram_tensor("wkT", [128, 9 * F], f16, kind="ExternalInput").ap()
    wvTe_d = nc.dram_tensor("wvTe", [128, 9 * 260], f16, kind="ExternalInput").ap()
    woT_d = nc.dram_tensor("woT", [128, 2 * DIM], f16, kind="ExternalInput").ap()
    mask_d = nc.dram_tensor("maskv", [S, 1], fp32, kind="ExternalInput").ap()
    CT_d = nc.dram_tensor("CT", [128, S], fp32, kind="ExternalInput").ap()
    SST_d = nc.dram_tensor("SST", [128, S], fp32, kind="ExternalInput").ap()
    tri_d = nc.dram_tensor("tri", [128, 128], f16, kind="ExternalInput").ap()
    negI_d = nc.dram_tensor("negI", [128, 128], f16, kind="ExternalInput").ap()
    ones32_d = nc.dram_tensor("ones32", [128, 64], f32r, kind="ExternalInput").ap()
    # out packed as [4 chunks][8 ntiles][128, 512] contiguous blocks
    out_d = nc.dram_tensor("outp", [32, 128, 512], fp32, kind="ExternalOutput").ap()
    if debug:
        dbg_qT = nc.dram_tensor("dbg_qT", [2, 128, S], f16, kind="ExternalOutput").ap()
        dbg_kT = nc.dram_tensor("dbg_kT", [2, 128, S], f16, kind="ExternalOutput").ap()
        dbg_v = nc.dram_tensor("dbg_v", [128, 16 * 260], f16, kind="ExternalOutput").ap()
        dbg_zn = nc.dram_tensor("dbg_zn", [2, 128, S], f16, kind="ExternalOutput").ap()

    with tile.TileContext(nc) as tc, ExitStack() as ctx:
        const = ctx.enter_context(tc.tile_pool(name="const", bufs=1))
        qk_pool = ctx.enter_context(tc.tile_pool(name="qk", bufs=1))
        v_pool = ctx.enter_context(tc.tile_pool(name="v", bufs=1))
        zn_pool = ctx.enter_context(tc.tile_pool(name="zn", bufs=1))

        # SBUF tiles (declared before DMA issue order below)
        wq_big = const.tile([128, 9 * F], f16, tag="wqbig", name="wqbig")
        wk_big = const.tile([128, 9 * F], f16, tag="wkbig", name="wkbig")
        wv_big = const.tile([128, 9 * 260], f16, tag="wvbig", name="wvbig")
        xT_big = const.tile([128, 4 * 4608], f16, tag="xTbig", name="xTbig")
        CT_s = const.tile([128, S], fp32, tag="CT", name="CT")
        SST_s = const.tile([128, S], fp32, tag="SST", name="SST")
        mask_s = const.tile([128, 16], fp32, tag="maskv", name="maskv")
        tri_s = const.tile([128, 128], f16, tag="tri", name="tri")
        negI_s = const.tile([128, 128], f16, tag="negI", name="negI")
        ones32_s = const.tile([128, 64], f32r, tag="ones32", name="ones32")
        wo_big = const.tile([128, 2 * DIM], f16, tag="wobig", name="wobig")
        woT_s = [wo_big[:, DIM * t:DIM * t + DIM] for t in range(2)]
        wq_s = [wq_big[:, F * d:F * d + F] for d in range(9)]
        wk_s = [wk_big[:, F * d:F * d + F] for d in range(9)]
        wv_s = [wv_big[:, 260 * d:260 * d + 260] for d in range(9)]

        # xT chunk-major: chunk j occupies columns [4608j, 4608(j+1))
        def xs(d, j):
            return xT_big[:, 4608 * j + 512 * d:4608 * j + 512 * d + 512]

        def xtok(d, t):
            # 128-token-tile t of chunk t//4
            o = 4608 * (t // 4) + 512 * d + 128 * (t % 4)
            return xT_big[:, o:o + 128]

        # DMA issue order tuned for earliest first matmul; bulk loads are
        # single contiguous-per-partition transfers split across SP and ACT
        # descriptor generators.
        nc.sync.dma_start(wq_big[:], wqT_d[:])
        nc.sync.dma_start(xT_big[:, 0:4608], xT_d[0])
        nc.scalar.dma_start(CT_s[:], CT_d[:])
        nc.scalar.dma_start(SST_s[:], SST_d[:])
        nc.sync.dma_start(wk_big[:], wkT_d[:])
        nc.sync.dma_start(wv_big[:], wvTe_d[:])
        nc.scalar.dma_start(mask_s[:], mask_d.rearrange("(t p) one -> p (t one)", p=128))
        nc.scalar.dma_start(tri_s[:], tri_d[:])
        nc.scalar.dma_start(negI_s[:], negI_d[:])
        nc.scalar.dma_start(ones32_s[:], ones32_d[:])
        nc.sync.dma_start(xT_big[:, 4608:2 * 4608], xT_d[1])
        nc.scalar.dma_start(wo_big[:], woT_d[:])
        nc.sync.dma_start(xT_big[:, 2 * 4608:3 * 4608], xT_d[2])
        nc.sync.dma_start(xT_big[:, 3 * 4608:4 * 4608], xT_d[3])

        qhatT = [qk_pool.tile([128, S], f16, tag=f"qhatT{t}", name=f"qhatT{t}")
                 for t in range(2)]
        khatT = [qk_pool.tile([128, S], f16, tag=f"khatT{t}", name=f"khatT{t}")
                 for t in range(2)]
        v_big = v_pool.tile([128, 16 * 260], f16, tag="vbig", name="vbig")
        v_sb = [v_big[:, 260 * t:260 * t + 260] for t in range(16)]
        znT = [zn_pool.tile([128, S], f16, tag=f"znT{t}", name=f"znT{t}")
               for t in range(2)]

        # unified psum pool: 4 x [128,1024] = all 8 banks
        psp = ctx.enter_context(tc.tile_pool(name="psp", bufs=4, space="PSUM"))
        rope_p = ctx.enter_context(tc.tile_pool(name="ropet", bufs=3))
        pb_p = ctx.enter_context(tc.tile_pool(name="pb", bufs=1))
        zr_p = ctx.enter_context(tc.tile_pool(name="zr", bufs=3))
        rr_p = ctx.enter_context(tc.tile_pool(name="rr", bufs=4))
        ob_p = ctx.enter_context(tc.tile_pool(name="ob", bufs=4))

        pb_big = pb_p.tile([128, 24 * 1024], f16, tag="pbbig", name="pbbig")

        def out_tile(j, n):
            cs = slice(512 * j, 512 * j + 512)
            ps = psp.tile([128, 1024], fp32, tag="ps", name="ps")
            for t in range(2):
                MM(ps[:, 0:512], woT_s[t][:, 128 * n:128 * n + 128],
                   znT[t][:, cs], start=(t == 0), stop=(t == 1))
            ot = ob_p.tile([128, 512], fp32, tag="osb", name="osb")
            nc.vector.tensor_copy(ot[:], ps[:, 0:512])
            nc.sync.dma_start(out_d[8 * j + n], ot[:])

        def qk_group(j, which, fc):
            cs = slice(512 * j, 512 * j + 512)
            w_s, dst = (wq_s, qhatT) if which == "q" else (wk_s, khatT)
            ps = psp.tile([128, 1024], fp32, tag="ps", name="ps")
            for d in range(9):
                MM(ps[:, 0:512], w_s[d][:, 128 * fc:128 * fc + 128],
                   xs(d, j), start=(d == 0), stop=(d == 8))
            sw = rope_p.tile([128, 512], fp32, tag="sw", name="sw")
            nc.vector.stream_shuffle(sw[:], ps[:, 0:512], _SHUF_MASK)
            t1 = rope_p.tile([128, 512], fp32, tag="t1", name="t1")
            nc.gpsimd.tensor_mul(t1[:], sw[:], SST_s[:, cs])
            t2 = rope_p.tile([128, 512], fp32, tag="t2", name="t2")
            nc.vector.tensor_mul(t2[:], ps[:, 0:512], CT_s[:, cs])
            nc.gpsimd.tensor_add(dst[fc][:, cs], t1[:], t2[:])

        def v_group(j, t):
            ps = psp.tile([128, 1024], fp32, tag="ps", name="ps")
            for d in range(9):
                MM(ps[:, 0:260], xtok(d, t),
                   wv_s[d][:], start=(d == 0), stop=(d == 8))
            nc.vector.tensor_scalar_mul(v_sb[t][:], ps[:, 0:260],
                                        mask_s[:, t:t + 1])

        def z_head(h, j, pts_h):
            """z matmuls + eager psum evac for head h, chunk j."""
            ni = 4 * j + 4
            zps = psp.tile([128, 1024], fp32, tag="ps", name="ps")
            for i in range(ni):
                d = i - 4 * j
                c0 = max(0, 128 * d)
                MM(zps[0:65, c0:512], v_sb[i][:, 65 * h:65 * h + 65],
                   pts_h[i][:, c0:512], start=(i == 0), stop=(i == ni - 1))
            zraw = zr_p.tile([65, 512], f32r, tag="zraw", name="zraw")
            nc.vector.tensor_copy(zraw[:], zps[0:65, 0:512])
            return (h, j, zraw)

        def norm_head(st):
            """PE-broadcast the softmax sums into psum, reciprocal on DVE,
            then normalize into znT."""
            h, j, zraw = st
            tH, rH = h // 2, 64 * (h % 2)
            cs = slice(512 * j, 512 * j + 512)
            rps = psp.tile([128, 1024], fp32, tag="ps", name="ps")
            MM(rps[0:64, 0:512], ones32_s[64:65, :], zraw[64:65, :],
               start=True, stop=True)
            rinv = rr_p.tile([64, 512], fp32, tag="rinv", name="rinv")
            nc.vector.reciprocal_approx_fast(rinv[:], rps[0:64, 0:512])
            if rH == 0:
                nc.vector.tensor_mul(znT[tH][0:64, cs], zraw[0:64, :], rinv[:])
            else:
                tmp = rr_p.tile([64, 512], fp32, tag="ztmp", name="ztmp")
                nc.vector.tensor_mul(tmp[:], zraw[0:64, :], rinv[:])
                nc.vector.tensor_copy(znT[tH][64:128, cs], tmp[:])

        def scores_head(h, j, seam_cb=None):
            """scores + exp for head h, chunk j; returns p-tile map.
            seam_cb, if given, is invoked between score pairs to emit
            independent PE work that smooths the PE/ACT pipeline."""
            ni = 4 * j + 4
            tH, rH = h // 2, 64 * (h % 2)
            pts = {}
            for ip in range((ni + 1) // 2):
                sps = psp.tile([128, 1024], fp32, tag="ps", name="ps")
                slot = 8 * (h % (zdelay + 1)) + ip
                pb = pb_big[:, 1024 * slot:1024 * slot + 1024]
                cmin = 1024
                masks = []
                for ii in range(2):
                    i = 2 * ip + ii
                    if i >= ni:
                        continue
                    d = i - 4 * j
                    c0 = max(0, 128 * d)
                    cmin = min(cmin, 512 * ii + c0)
                    if d >= 0:
                        masks.append(512 * ii + 128 * d)
                    MM(sps[:, 512 * ii + c0:512 * ii + 512],
                       khatT[tH][rH:rH + 64, 128 * i:128 * i + 128],
                       qhatT[tH][rH:rH + 64, 512 * j + c0:512 * j + 512],
                       start=True, stop=(d < 0))
                    pts[i] = pb[:, 512 * ii:512 * ii + 512]
                for cc in masks:
                    # causal mask on PE: accumulate (-60000*I)^T @ tril01 so
                    # exp underflows to exactly 0 above the diagonal
                    MM(sps[:, cc:cc + 128], negI_s[:], tri_s[:],
                       start=False, stop=True)
                hi = 1024 if 2 * ip + 1 < ni else 512
                nc.scalar.activation(pb[:, cmin:hi], sps[:, cmin:hi], AF.Exp)
                if seam_cb is not None and ip % 2 == 1:
                    seam_cb()
            return pts

        # ---- prologue: projections for chunk 0 ----
        for fc in range(2):
            qk_group(0, "q", fc)
        for fc in range(2):
            qk_group(0, "k", fc)
        for t in range(4):
            v_group(0, t)

        pending = [None]      # deferred h3 norm from the previous chunk
        for j in range(4):
            # seam work: out_proj(j-1) tiles + proj/v groups for chunk j+1,
            # interleaved between attention heads so the PE never starves
            # while ACT drains exp.
            seams = []
            if j < 3:
                seams += [(qk_group, (j + 1, "q", 0)),
                          (qk_group, (j + 1, "k", 0)),
                          (qk_group, (j + 1, "q", 1)),
                          (qk_group, (j + 1, "k", 1))]
            # out tiles are deferred by TWO chunks: chunk 3 is ACT(exp)-bound
            # and otherwise has little independent seam work for the PE
            if j >= 2:
                seams += [(out_tile, (j - 2, n)) for n in range(8)]
            if j == 3:
                seams += [(out_tile, (2, n)) for n in range(8)]
            if j < 3:
                seams += [(v_group, (j + 1, t)) for t in range(4 * j + 4,
                                                               4 * j + 8)]
            if pending[0] is not None:
                st = pending[0]
                pending[0] = None
                seams = [(norm_head, (st,))] + seams
            seams = seams[::-1]          # pop() from the front of the list
            per_seam = max(1, (len(seams) + HPC - 1) // HPC)

            zdelay = 1
            hq = []              # (h, pts) awaiting z
            zq = []
            def seam_cb():
                if seams:
                    fn, args = seams.pop()
                    fn(*args)

            # head order alternates parity and ends on an even head so the
            # tail-critical final normalize writes znT directly (no copy)
            for h in (1, 0, 3, 2):
                hq.append((h, scores_head(h, j, seam_cb if j >= 1 else None)))
                if j < 2:
                    for _ in range(per_seam):
                        seam_cb()
                if len(hq) > zdelay:
                    ph, ppts = hq.pop(0)
                    zq.append(z_head(ph, j, ppts))
                if len(zq) >= 2:
                    norm_head(zq.pop(0))
            while hq:
                ph, ppts = hq.pop(0)
                zq.append(z_head(ph, j, ppts))
            while seams:
                fn, args = seams.pop()
                fn(*args)
            while len(zq) > 1:
                norm_head(zq.pop(0))
            pending[0] = zq.pop(0)

        norm_head(pending[0])
        out_proj3 = [out_tile(3, n) for n in range(8)]

        if debug:
            for t in range(2):
                nc.sync.dma_start(dbg_qT[t], qhatT[t][:])
                nc.sync.dma_start(dbg_kT[t], khatT[t][:])
                nc.sync.dma_start(dbg_zn[t], znT[t][:])
            nc.sync.dma_start(dbg_v[:], v_big[:])

    nc.finalize()
    return nc


_NC_CACHE = {}


def kernel(x, attn_mask, wq_w, wq_b, wk_w, wk_b, wv_w, wv_b, wo_w, wo_b):
    from concourse.bass_utils import run_bass_kernel_spmd

    x = np.asarray(x, np.float32)
    attn_mask = np.asarray(attn_mask)
    wq_w = np.asarray(wq_w, np.float32); wq_b = np.asarray(wq_b, np.float32)
    wk_w = np.asarray(wk_w, np.float32); wk_b = np.asarray(wk_b, np.float32)
    wv_w = np.asarray(wv_w, np.float32); wv_b = np.asarray(wv_b, np.float32)
    wo_w = np.asarray(wo_w, np.float32); wo_b = np.asarray(wo_b, np.float32)

    CT, SST = _rope_tables()
    tri01 = np.tril(np.ones((128, 128), np.float16), -1)

    hp = _head_perm()
    perm = np.concatenate([64 * h + hp for h in range(HPC)])  # [256]

    in_maps = []
    for c in range(NC):
        b, g = c // 4, c % 4
        fs = slice(F * g, F * g + F)
        wq = wq_w[fs][perm] / np.float32(8.0)
        wk = wk_w[fs][perm]
        qb = wq_b[fs][perm] / np.float32(8.0)
        kb = wk_b[fs][perm]
        wqTe = np.zeros((DIM + 128, F), np.float16)
        wqTe[0:DIM] = wq.T.astype(np.float16)
        wqTe[DIM] = qb.astype(np.float16)
        wkTe = np.zeros((DIM + 128, F), np.float16)
        wkTe[0:DIM] = wk.T.astype(np.float16)
        wkTe[DIM] = kb.astype(np.float16)
        wv = wv_w[fs]
        vb = wv_b[fs]
        wvTe = np.zeros((DIM + 128, 260), np.float16)
        for h in range(HPC):
            wvTe[0:DIM, 65 * h:65 * h + 64] = wv[64 * h:64 * h + 64].T.astype(np.float16)
            wvTe[DIM, 65 * h:65 * h + 64] = vb[64 * h:64 * h + 64].astype(np.float16)
            wvTe[DIM, 65 * h + 64] = 1.0
        xTe = np.zeros((DIM + 128, S), np.float16)
        xTe[0:DIM] = x[b].T.astype(np.float16)
        xTe[DIM] = 1.0
        # pack xT chunk-major: [4 chunks][128 partitions][9 dtiles * 512]
        xTb = np.ascontiguousarray(
            xTe.reshape(9, 128, 4, 512).transpose(2, 1, 0, 3)
        ).reshape(4, 128, 9 * 512)
        woTc = np.ascontiguousarray(wo_w[:, fs].T).astype(np.float16)
        in_maps.append({
            "xT": xTb,
            "wqT": np.ascontiguousarray(
                wqTe.reshape(9, 128, F).transpose(1, 0, 2)).reshape(128, 9 * F),
            "wkT": np.ascontiguousarray(
                wkTe.reshape(9, 128, F).transpose(1, 0, 2)).reshape(128, 9 * F),
            "wvTe": np.ascontiguousarray(
                wvTe.reshape(9, 128, 260).transpose(1, 0, 2)).reshape(128, 9 * 260),
            "woT": np.ascontiguousarray(
                woTc.reshape(2, 128, DIM).transpose(1, 0, 2)).reshape(128, 2 * DIM),
            "maskv": attn_mask[b].astype(np.float32).reshape(S, 1).copy(),
            "CT": CT, "SST": SST, "tri": tri01,
            "negI": (np.eye(128) * -60000.0).astype(np.float16),
            "ones32": np.ones((128, 64), np.float32),
        })

    if "nc" not in _NC_CACHE:
        _NC_CACHE["nc"] = _build_program()
    res = run_bass_kernel_spmd(_NC_CACHE["nc"], in_maps, core_ids=list(range(NC)))
    globals()["LAST_RESULTS"] = res

    out = np.zeros((B, DIM, S), np.float32)
    for c in range(NC):
        # outp [32,128,512] = [4 chunks][8 ntiles][128,512] -> [1024, 2048]
        o = res.results[c]["outp"].reshape(4, 8, 128, 512)
        out[c // 4] += o.transpose(1, 2, 0, 3).reshape(DIM, S)
    out = out.transpose(0, 2, 1) + wo_b[None, None, :]
    return np.ascontiguousarray(out)


if __name__ == "__main__":
    rng = np.random.default_rng(0)
    ins = {
        "x": rng.standard_normal((B, S, DIM)).astype(np.float32),
        "attn_mask": np.ones((B, S), bool),
    }
    for n in ["wq", "wk", "wv", "wo"]:
        ins[n + "_w"] = (rng.standard_normal((DIM, DIM)).astype(np.float32) / 32.0)
        ins[n + "_b"] = rng.standard_normal(DIM).astype(np.float32) * 0.01
    o = kernel(**ins)
    print("ran", o.shape, o.dtype)


# revision 53
# speedup vs baseline: 1.2220x; 1.2220x over previous
"""AttentionWithRope Trainium2 Bass kernel (v3, fp16 datapath, pipelined).

Sharding: 8 cores = 2 batches x 4 head-groups (4 heads / 256 features).
Per core: fp16 projections (biases folded in via the x ones-row), RoPE via
stream_shuffle (head features host-permuted so each rotate pair sits +-16
partitions apart inside a 32-partition shuffle window), causal attention
with the diagonal mask applied on the PE (accumulating (-60000*I)^T @
tril01 so exp underflows to exactly 0), transposed output projection.
Host sums the 4 partial outT per batch and adds wo_b.

Pipelining: one unified [128,1024] PSUM pool (4 bufs = all 8 banks) lets
the PE run ahead; z matmuls are delayed by one head so exp(h-1) hides
under scores(h); projection/output-projection groups for adjacent chunks
are emitted into the seams between attention heads so the PE never
starves while ACT drains exp (and stays out of the slow p-state). The
softmax normalization avoids the 3.3us DVE reciprocal on the critical
path: z psum is evacuated to SBUF (f32r), the sums row is broadcast
across 64 partitions by a tiny fp32r PE matmul (ones^T @ sums), and a
fast approximate reciprocal + multiply produce normalized fp16 zn. The
causal mask is applied on the PE by accumulating (-60000*I)^T @ tril01
into the diagonal score blocks (exp underflows to exactly 0). Seam
items are ordered q/k-projections first (they gate the next chunk's
scores), then out-projection tiles, then v; at j>=1 they are emitted at
score-pair granularity. x/weights/out use packed contiguous-per-partition DRAM layouts
(one descriptor-cheap DMA each), issued across both the SP and ACT
descriptor generators.
"""

import numpy as np
from contextlib import ExitStack

DIM, HEADS, HD = 1024, 16, 64
B, S = 2, 2048
NC = 8
HPC = 4          # heads per core
F = HPC * HD     # 256 features per core
ROPE_BASE = 10000.0


def _head_perm():
    # within-head feature order: [x1[0:16], x2[0:16], x1[16:32], x2[16:32]]
    return np.concatenate([np.arange(0, 16), np.arange(32, 48),
                           np.arange(16, 32), np.arange(48, 64)])


def _rope_tables():
    """CT/SST [128, S] fp32 in the permuted layout (2 heads per tile)."""
    theta = ROPE_BASE ** (-np.arange(0, HD, 2, dtype=np.float32) / HD)  # [32]
    pos = np.arange(S, dtype=np.float32)
    ang = pos[:, None] * theta[None, :]            # [S, 32]
    cos, sin = np.cos(ang).T, np.sin(ang).T        # [32, S]
    cA, cB = cos[0:16], cos[16:32]
    sA, sB = sin[0:16], sin[16:32]
    ct_head = np.concatenate([cA, cA, cB, cB], 0)              # [64, S]
    sst_head = np.concatenate([-sA, sA, -sB, sB], 0)           # [64, S]
    CT = np.concatenate([ct_head, ct_head], 0).astype(np.float32)    # [128, S]
    SST = np.concatenate([sst_head, sst_head], 0).astype(np.float32)
    return CT, SST


_SHUF_MASK = list(range(16, 32)) + list(range(0, 16))


def _build_program(debug=False):
    import concourse.bass as bass
    import concourse.mybir as mybir
    import concourse.tile as tile
    from concourse import bacc

    fp32 = mybir.dt.float32
    f32r = mybir.dt.float32r
    f16 = mybir.dt.float16
    AF = mybir.ActivationFunctionType

    nc = bacc.Bacc("TRN2", target_bir_lowering=False, num_devices=NC)

    from bass_rust import add_dep_helper as _adh
    _prev_mm = [None]

    def MM(*args, **kw):
        bi = nc.tensor.matmul(*args, **kw)
        if _prev_mm[0] is not None:
            _adh(bi.ins, _prev_mm[0].ins, sync=False, reason="pe-order")
        _prev_mm[0] = bi
        return bi

    # ---- DRAM I/O ----
    # xT packed as [4 chunks][128, 9*512]; weights packed partition-major
    xT_d = nc.dram_tensor("xT", [4, 128, 9 * 512], f16, kind="ExternalInput").ap()
    wqT_d = nc.dram_tensor("wqT", [128, 9 * F], f16, kind="ExternalInput").ap()
    wkT_d = nc.dram_tensor("wkT", [128, 9 * F], f16, kind="ExternalInput").ap()
    wvTe_d = nc.dram_tensor("wvTe", [128, 9 * 260], f16, kind="ExternalInput").ap()
    woT_d = nc.dram_tensor("woT", [128, 2 * DIM], f16, kind="ExternalInput").ap()
    mask_d = nc.dram_tensor("maskv", [S, 1], fp32, kind="ExternalInput").ap()
    CT_d = nc.dram_tensor("CT", [128, S], fp32, kind="ExternalInput").ap()
    SST_d = nc.dram_tensor("SST", [128, S], fp32, kind="ExternalInput").ap()
    tri_d = nc.dram_tensor("tri", [128, 128], f16, kind="ExternalInput").ap()
    negI_d = nc.dram_tensor("negI", [128, 128], f16, kind="ExternalInput").ap()
    ones32_d = nc.dram_tensor("ones32", [128, 64], f32r, kind="ExternalInput").ap()
    # out packed as [4 chunks][8 ntiles][128, 512] contiguous blocks
    out_d = nc.dram_tensor("outp", [32, 128, 512], fp32, kind="ExternalOutput").ap()
    if debug:
        dbg_qT = nc.dram_tensor("dbg_qT", [2, 128, S], f16, kind="ExternalOutput").ap()
        dbg_kT = nc.dram_tensor("dbg_kT", [2, 128, S], f16, kind="ExternalOutput").ap()
        dbg_v = nc.dram_tensor("dbg_v", [128, 16 * 260], f16, kind="ExternalOutput").ap()
        dbg_zn = nc.dram_tensor("dbg_zn", [2, 128, S], f16, kind="ExternalOutput").ap()

    with tile.TileContext(nc) as tc, ExitStack() as ctx:
        const = ctx.enter_context(tc.tile_pool(name="const", bufs=1))
        qk_pool = ctx.enter_context(tc.tile_pool(name="qk", bufs=1))
        v_pool = ctx.enter_context(tc.tile_pool(name="v", bufs=1))
        zn_pool = ctx.enter_context(tc.tile_pool(name="zn", bufs=1))

        # SBUF tiles (declared before DMA issue order below)
        wq_big = const.tile([128, 9 * F], f16, tag="wqbig", name="wqbig")
        wk_big = const.tile([128, 9 * F], f16, tag="wkbig", name="wkbig")
        wv_big = const.tile([128, 9 * 260], f16, tag="wvbig", name="wvbig")
        xT_big = const.tile([128, 4 * 4608], f16, tag="xTbig", name="xTbig")
        CT_s = const.tile([128, S], fp32, tag="CT", name="CT")
        SST_s = const.tile([128, S], fp32, tag="SST", name="SST")
        mask_s = const.tile([128, 16], fp32, tag="maskv", name="maskv")
        tri_s = const.tile([128, 128], f16, tag="tri", name="tri")
        negI_s = const.tile([128, 128], f16, tag="negI", name="negI")
        ones32_s = const.tile([128, 64], f32r, tag="ones32", name="ones32")
        wo_big = const.tile([128, 2 * DIM], f16, tag="wobig", name="wobig")
        woT_s = [wo_big[:, DIM * t:DIM * t + DIM] for t in range(2)]
        wq_s = [wq_big[:, F * d:F * d + F] for d in range(9)]
        wk_s = [wk_big[:, F * d:F * d + F] for d in range(9)]
        wv_s = [wv_big[:, 260 * d:260 * d + 260] for d in range(9)]

        # xT chunk-major: chunk j occupies columns [4608j, 4608(j+1))
        def xs(d, j):
            return xT_big[:, 4608 * j + 512 * d:4608 * j + 512 * d + 512]

        def xtok(d, t):
            # 128-token-tile t of chunk t//4
            o = 4608 * (t // 4) + 512 * d + 128 * (t % 4)
            return xT_big[:, o:o + 128]

        # DMA issue order tuned for earliest first matmul; bulk loads are
        # single contiguous-per-partition transfers split across SP and ACT
        # descriptor generators.
        nc.sync.dma_start(wq_big[:], wqT_d[:])
        nc.sync.dma_start(xT_big[:, 0:4608], xT_d[0])
        nc.scalar.dma_start(CT_s[:], CT_d[:])
        nc.scalar.dma_start(SST_s[:], SST_d[:])
        nc.sync.dma_start(wk_big[:], wkT_d[:])
        nc.sync.dma_start(wv_big[:], wvTe_d[:])
        nc.scalar.dma_start(mask_s[:], mask_d.rearrange("(t p) one -> p (t one)", p=128))
        nc.scalar.dma_start(tri_s[:], tri_d[:])
        nc.scalar.dma_start(negI_s[:], negI_d[:])
        nc.scalar.dma_start(ones32_s[:], ones32_d[:])
        nc.sync.dma_start(xT_big[:, 4608:2 * 4608], xT_d[1])
        nc.scalar.dma_start(wo_big[:], woT_d[:])
        nc.sync.dma_start(xT_big[:, 2 * 4608:3 * 4608], xT_d[2])
        nc.sync.dma_start(xT_big[:, 3 * 4608:4 * 4608], xT_d[3])

        qhatT = [qk_pool.tile([128, S], f16, tag=f"qhatT{t}", name=f"qhatT{t}")
                 for t in range(2)]
        khatT = [qk_pool.tile([128, S], f16, tag=f"khatT{t}", name=f"khatT{t}")
                 for t in range(2)]
        v_big = v_pool.tile([128, 16 * 260], f16, tag="vbig", name="vbig")
        v_sb = [v_big[:, 260 * t:260 * t + 260] for t in range(16)]
        znT = [zn_pool.tile([128, S], f16, tag=f"znT{t}", name=f"znT{t}")
               for t in range(2)]

        # unified psum pool: 4 x [128,1024] = all 8 banks
        psp = ctx.enter_context(tc.tile_pool(name="psp", bufs=4, space="PSUM"))
        rope_p = ctx.enter_context(tc.tile_pool(name="ropet", bufs=3))
        pb_p = ctx.enter_context(tc.tile_pool(name="pb", bufs=1))
        zr_p = ctx.enter_context(tc.tile_pool(name="zr", bufs=3))
        rr_p = ctx.enter_context(tc.tile_pool(name="rr", bufs=4))
        ob_p = ctx.enter_context(tc.tile_pool(name="ob", bufs=4))

        pb_big = pb_p.tile([128, 24 * 1024], f16, tag="pbbig", name="pbbig")

        def out_tile(j, n):
            cs = slice(512 * j, 512 * j + 512)
            ps = psp.tile([128, 1024], fp32, tag="ps", name="ps")
            for t in range(2):
                MM(ps[:, 0:512], woT_s[t][:, 128 * n:128 * n + 128],
                   znT[t][:, cs], start=(t == 0), stop=(t == 1))
            ot = ob_p.tile([128, 512], fp32, tag="osb", name="osb")
            nc.vector.tensor_copy(ot[:], ps[:, 0:512])
            nc.sync.dma_start(out_d[8 * j + n], ot[:])

        def qk_group(j, which, fc):
            cs = slice(512 * j, 512 * j + 512)
            w_s, dst = (wq_s, qhatT) if which == "q" else (wk_s, khatT)
            ps = psp.tile([128, 1024], fp32, tag="ps", name="ps")
            for d in range(9):
                MM(ps[:, 0:512], w_s[d][:, 128 * fc:128 * fc + 128],
                   xs(d, j), start=(d == 0), stop=(d == 8))
            sw = rope_p.tile([128, 512], fp32, tag="sw", name="sw")
            nc.vector.stream_shuffle(sw[:], ps[:, 0:512], _SHUF_MASK)
            t1 = rope_p.tile([128, 512], fp32, tag="t1", name="t1")
            nc.gpsimd.tensor_mul(t1[:], sw[:], SST_s[:, cs])
            t2 = rope_p.tile([128, 512], fp32, tag="t2", name="t2")
            nc.vector.tensor_mul(t2[:], ps[:, 0:512], CT_s[:, cs])
            nc.gpsimd.tensor_add(dst[fc][:, cs], t1[:], t2[:])

        def v_group(j, t):
            ps = psp.tile([128, 1024], fp32, tag="ps", name="ps")
            for d in range(9):
                MM(ps[:, 0:260], xtok(d, t),
                   wv_s[d][:], start=(d == 0), stop=(d == 8))
            nc.vector.tensor_scalar_mul(v_sb[t][:], ps[:, 0:260],
                                        mask_s[:, t:t + 1])

        def z_head(h, j, pts_h):
            """z matmuls + eager psum evac for head h, chunk j."""
            ni = 4 * j + 4
            zps = psp.tile([128, 1024], fp32, tag="ps", name="ps")
            for i in range(ni):
                d = i - 4 * j
                c0 = max(0, 128 * d)
                MM(zps[0:65, c0:512], v_sb[i][:, 65 * h:65 * h + 65],
                   pts_h[i][:, c0:512], start=(i == 0), stop=(i == ni - 1))
            zraw = zr_p.tile([65, 512], f32r, tag="zraw", name="zraw")
            nc.vector.tensor_copy(zraw[:], zps[0:65, 0:512])
            return (h, j, zraw)

        def norm_head(st):
            """PE-broadcast the softmax sums into psum, reciprocal on DVE,
            then normalize into znT."""
            h, j, zraw = st
            tH, rH = h // 2, 64 * (h % 2)
            cs = slice(512 * j, 512 * j + 512)
            rps = psp.tile([128, 1024], fp32, tag="ps", name="ps")
            MM(rps[0:64, 0:512], ones32_s[64:65, :], zraw[64:65, :],
               start=True, stop=True)
            rinv = rr_p.tile([64, 512], fp32, tag="rinv", name="rinv")
            nc.vector.reciprocal_approx_fast(rinv[:], rps[0:64, 0:512])
            if rH == 0:
                nc.vector.tensor_mul(znT[tH][0:64, cs], zraw[0:64, :], rinv[:])
            else:
                tmp = rr_p.tile([64, 512], fp32, tag="ztmp", name="ztmp")
                nc.vector.tensor_mul(tmp[:], zraw[0:64, :], rinv[:])
                nc.vector.tensor_copy(znT[tH][64:128, cs], tmp[:])

        def scores_head(h, j, seam_cb=None):
            """scores + exp for head h, chunk j; returns p-tile map.
            seam_cb, if given, is invoked between score pairs to emit
            independent PE work that smooths the PE/ACT pipeline."""
            ni = 4 * j + 4
            tH, rH = h // 2, 64 * (h % 2)
            pts = {}
            for ip in range((ni + 1) // 2):
                sps = psp.tile([128, 1024], fp32, tag="ps", name="ps")
                slot = 8 * (h % (zdelay + 1)) + ip
                pb = pb_big[:, 1024 * slot:1024 * slot + 1024]
                cmin = 1024
                masks = []
                for ii in range(2):
                    i = 2 * ip + ii
                    if i >= ni:
                        continue
                    d = i - 4 * j
                    c0 = max(0, 128 * d)
                    cmin = min(cmin, 512 * ii + c0)
                    if d >= 0:
                        masks.append(512 * ii + 128 * d)
                    MM(sps[:, 512 * ii + c0:512 * ii + 512],
                       khatT[tH][rH:rH + 64, 128 * i:128 * i + 128],
                       qhatT[tH][rH:rH + 64, 512 * j + c0:512 * j + 512],
                       start=True, stop=(d < 0))
                    pts[i] = pb[:, 512 * ii:512 * ii + 512]
                for cc in masks:
                    # causal mask on PE: accumulate (-60000*I)^T @ tril01 so
                    # exp underflows to exactly 0 above the diagonal
                    MM(sps[:, cc:cc + 128], negI_s[:], tri_s[:],
                       start=False, stop=True)
                hi = 1024 if 2 * ip + 1 < ni else 512
                nc.scalar.activation(pb[:, cmin:hi], sps[:, cmin:hi], AF.Exp)
                if seam_cb is not None and ip % 2 == 1:
                    seam_cb()
            return pts

        # ---- prologue: projections for chunk 0 ----
        for fc in range(2):
            qk_group(0, "q", fc)
        for fc in range(2):
            qk_group(0, "k", fc)
        for t in range(4):
            v_group(0, t)

        pending = [None]      # deferred h3 norm from the previous chunk
        for j in range(4):
            # seam work: out_proj(j-1) tiles + proj/v groups for chunk j+1,
            # interleaved between attention heads so the PE never starves
            # while ACT drains exp.
            seams = []
            if j < 3:
                seams += [(qk_group, (j + 1, "q", 0)),
                          (qk_group, (j + 1, "k", 0)),
                          (qk_group, (j + 1, "q", 1)),
                          (qk_group, (j + 1, "k", 1))]
            # out tiles are deferred by TWO chunks: chunk 3 is ACT(exp)-bound
            # and otherwise has little independent seam work for the PE
            if j >= 2:
                seams += [(out_tile, (j - 2, n)) for n in range(8)]
            if j == 3:
                seams += [(out_tile, (2, n)) for n in range(8)]
            if j < 3:
                seams += [(v_group, (j + 1, t)) for t in range(4 * j + 4,
                                                               4 * j + 8)]
            if pending[0] is not None:
                st = pending[0]
                pending[0] = None
                seams = [(norm_head, (st,))] + seams
            seams = seams[::-1]          # pop() from the front of the list
            per_seam = max(1, (len(seams) + HPC - 1) // HPC)

            zdelay = 1
            hq = []              # (h, pts) awaiting z
            zq = []
            def seam_cb():
                if seams:
                    fn, args = seams.pop()
                    fn(*args)

            for h in range(HPC):
                hq.append((h, scores_head(h, j, seam_cb if j >= 1 else None)))
                if j < 2:
                    for _ in range(per_seam):
                        seam_cb()
                if len(hq) > zdelay:
                    ph, ppts = hq.pop(0)
                    zq.append(z_head(ph, j, ppts))
                if len(zq) >= 2:
                    norm_head(zq.pop(0))
            while hq:
                ph, ppts = hq.pop(0)
                zq.append(z_head(ph, j, ppts))
            while seams:
                fn, args = seams.pop()
                fn(*args)
            while len(zq) > 1:
                norm_head(zq.pop(0))
            pending[0] = zq.pop(0)

        norm_head(pending[0])
        out_proj3 = [out_tile(3, n) for n in range(8)]

        if debug:
            for t in range(2):
                nc.sync.dma_start(dbg_qT[t], qhatT[t][:])
                nc.sync.dma_start(dbg_kT[t], khatT[t][:])
                nc.sync.dma_start(dbg_zn[t], znT[t][:])
            nc.sync.dma_start(dbg_v[:], v_big[:])

    nc.finalize()
    return nc


_NC_CACHE = {}


def kernel(x, attn_mask, wq_w, wq_b, wk_w, wk_b, wv_w, wv_b, wo_w, wo_b):
    from concourse.bass_utils import run_bass_kernel_spmd

    x = np.asarray(x, np.float32)
    attn_mask = np.asarray(attn_mask)
    wq_w = np.asarray(wq_w, np.float32); wq_b = np.asarray(wq_b, np.float32)
    wk_w = np.asarray(wk_w, np.float32); wk_b = np.asarray(wk_b, np.float32)
    wv_w = np.asarray(wv_w, np.float32); wv_b = np.asarray(wv_b, np.float32)
    wo_w = np.asarray(wo_w, np.float32); wo_b = np.asarray(wo_b, np.float32)

    CT, SST = _rope_tables()
    tri01 = np.tril(np.ones((128, 128), np.float16), -1)

    hp = _head_perm()
    perm = np.concatenate([64 * h + hp for h in range(HPC)])  # [256]

    in_maps = []
    for c in range(NC):
        b, g = c // 4, c % 4
        fs = slice(F * g, F * g + F)
        wq = wq_w[fs][perm] / np.float32(8.0)
        wk = wk_w[fs][perm]
        qb = wq_b[fs][perm] / np.float32(8.0)
        kb = wk_b[fs][perm]
        wqTe = np.zeros((DIM + 128, F), np.float16)
        wqTe[0:DIM] = wq.T.astype(np.float16)
        wqTe[DIM] = qb.astype(np.float16)
        wkTe = np.zeros((DIM + 128, F), np.float16)
        wkTe[0:DIM] = wk.T.astype(np.float16)
        wkTe[DIM] = kb.astype(np.float16)
        wv = wv_w[fs]
        vb = wv_b[fs]
        wvTe = np.zeros((DIM + 128, 260), np.float16)
        for h in range(HPC):
            wvTe[0:DIM, 65 * h:65 * h + 64] = wv[64 * h:64 * h + 64].T.astype(np.float16)
            wvTe[DIM, 65 * h:65 * h + 64] = vb[64 * h:64 * h + 64].astype(np.float16)
            wvTe[DIM, 65 * h + 64] = 1.0
        xTe = np.zeros((DIM + 128, S), np.float16)
        xTe[0:DIM] = x[b].T.astype(np.float16)
        xTe[DIM] = 1.0
        # pack xT chunk-major: [4 chunks][128 partitions][9 dtiles * 512]
        xTb = np.ascontiguousarray(
            xTe.reshape(9, 128, 4, 512).transpose(2, 1, 0, 3)
        ).reshape(4, 128, 9 * 512)
        woTc = np.ascontiguousarray(wo_w[:, fs].T).astype(np.float16)
        in_maps.append({
            "xT": xTb,
            "wqT": np.ascontiguousarray(
                wqTe.reshape(9, 128, F).transpose(1, 0, 2)).reshape(128, 9 * F),
            "wkT": np.ascontiguousarray(
                wkTe.reshape(9, 128, F).transpose(1, 0, 2)).reshape(128, 9 * F),
            "wvTe": np.ascontiguousarray(
                wvTe.reshape(9, 128, 260).transpose(1, 0, 2)).reshape(128, 9 * 260),
            "woT": np.ascontiguousarray(
                woTc.reshape(2, 128, DIM).transpose(1, 0, 2)).reshape(128, 2 * DIM),
            "maskv": attn_mask[b].astype(np.float32).reshape(S, 1).copy(),
            "CT": CT, "SST": SST, "tri": tri01,
            "negI": (np.eye(128) * -60000.0).astype(np.float16),
            "ones32": np.ones((128, 64), np.float32),
        })

    if "nc" not in _NC_CACHE:
        _NC_CACHE["nc"] = _build_program()
    res = run_bass_kernel_spmd(_NC_CACHE["nc"], in_maps, core_ids=list(range(NC)))
    globals()["LAST_RESULTS"] = res

    out = np.zeros((B, DIM, S), np.float32)
    for c in range(NC):
        # outp [32,128,512] = [4 chunks][8 ntiles][128,512] -> [1024, 2048]
        o = res.results[c]["outp"].reshape(4, 8, 128, 512)
        out[c // 4] += o.transpose(1, 2, 0, 3).reshape(DIM, S)
    out = out.transpose(0, 2, 1) + wo_b[None, None, :]
    return np.ascontiguousarray(out)


if __name__ == "__main__":
    rng = np.random.default_rng(0)
    ins = {
        "x": rng.standard_normal((B, S, DIM)).astype(np.float32),
        "attn_mask": np.ones((B, S), bool),
    }
    for n in ["wq", "wk", "wv", "wo"]:
        ins[n + "_w"] = (rng.standard_normal((DIM, DIM)).astype(np.float32) / 32.0)
        ins[n + "_b"] = rng.standard_normal(DIM).astype(np.float32) * 0.01
    o = kernel(**ins)
    print("ran", o.shape, o.dtype)


# revision 54
# speedup vs baseline: 1.2285x; 1.0053x over previous
"""AttentionWithRope Trainium2 Bass kernel (v3, fp16 datapath, pipelined).

Sharding: 8 cores = 2 batches x 4 head-groups (4 heads / 256 features).
Per core: fp16 projections (biases folded in via the x ones-row), RoPE via
stream_shuffle (head features host-permuted so each rotate pair sits +-16
partitions apart inside a 32-partition shuffle window), causal attention
with the diagonal mask applied on the PE (accumulating (-60000*I)^T @
tril01 so exp underflows to exactly 0), transposed output projection.
Host sums the 4 partial outT per batch and adds wo_b.

Pipelining: one unified [128,1024] PSUM pool (4 bufs = all 8 banks) lets
the PE run ahead; z matmuls are delayed by one head so exp(h-1) hides
under scores(h); projection/output-projection groups for adjacent chunks
are emitted into the seams between attention heads so the PE never
starves while ACT drains exp (and stays out of the slow p-state). The
softmax normalization avoids the 3.3us DVE reciprocal on the critical
path: z psum is evacuated to SBUF (f32r), the sums row is broadcast
across 64 partitions by a tiny fp32r PE matmul (ones^T @ sums), and a
fast approximate reciprocal + multiply produce normalized fp16 zn. The
causal mask is applied on the PE by accumulating (-60000*I)^T @ tril01
into the diagonal score blocks (exp underflows to exactly 0). Seam
items are ordered q/k-projections first (they gate the next chunk's
scores), then out-projection tiles, then v; at j>=1 they are emitted at
score-pair granularity. x/weights/out use packed contiguous-per-partition DRAM layouts
(one descriptor-cheap DMA each), issued across both the SP and ACT
descriptor generators.
"""

import numpy as np
from contextlib import ExitStack

DIM, HEADS, HD = 1024, 16, 64
B, S = 2, 2048
NC = 8
HPC = 4          # heads per core
F = HPC * HD     # 256 features per core
ROPE_BASE = 10000.0


def _head_perm():
    # within-head feature order: [x1[0:16], x2[0:16], x1[16:32], x2[16:32]]
    return np.concatenate([np.arange(0, 16), np.arange(32, 48),
                           np.arange(16, 32), np.arange(48, 64)])


def _rope_tables():
    """CT/SST [128, S] fp32 in the permuted layout (2 heads per tile)."""
    theta = ROPE_BASE ** (-np.arange(0, HD, 2, dtype=np.float32) / HD)  # [32]
    pos = np.arange(S, dtype=np.float32)
    ang = pos[:, None] * theta[None, :]            # [S, 32]
    cos, sin = np.cos(ang).T, np.sin(ang).T        # [32, S]
    cA, cB = cos[0:16], cos[16:32]
    sA, sB = sin[0:16], sin[16:32]
    ct_head = np.concatenate([cA, cA, cB, cB], 0)              # [64, S]
    sst_head = np.concatenate([-sA, sA, -sB, sB], 0)           # [64, S]
    CT = np.concatenate([ct_head, ct_head], 0).astype(np.float32)    # [128, S]
    SST = np.concatenate([sst_head, sst_head], 0).astype(np.float32)
    return CT, SST


_SHUF_MASK = list(range(16, 32)) + list(range(0, 16))


def _build_program(debug=False):
    import concourse.bass as bass
    import concourse.mybir as mybir
    import concourse.tile as tile
    from concourse import bacc

    fp32 = mybir.dt.float32
    f32r = mybir.dt.float32r
    f16 = mybir.dt.float16
    AF = mybir.ActivationFunctionType

    nc = bacc.Bacc("TRN2", target_bir_lowering=False, num_devices=NC)

    from bass_rust import add_dep_helper as _adh
    _prev_mm = [None]

    def MM(*args, **kw):
        bi = nc.tensor.matmul(*args, **kw)
        if _prev_mm[0] is not None:
            _adh(bi.ins, _prev_mm[0].ins, sync=False, reason="pe-order")
        _prev_mm[0] = bi
        return bi

    # ---- DRAM I/O ----
    # xT packed as [4 chunks][128, 9*512]; weights packed partition-major
    xT_d = nc.dram_tensor("xT", [4, 128, 9 * 512], f16, kind="ExternalInput").ap()
    wqT_d = nc.dram_tensor("wqT", [128, 9 * F], f16, kind="ExternalInput").ap()
    wkT_d = nc.dram_tensor("wkT", [128, 9 * F], f16, kind="ExternalInput").ap()
    wvTe_d = nc.dram_tensor("wvTe", [128, 9 * 260], f16, kind="ExternalInput").ap()
    woT_d = nc.dram_tensor("woT", [128, 2 * DIM], f16, kind="ExternalInput").ap()
    mask_d = nc.dram_tensor("maskv", [S, 1], fp32, kind="ExternalInput").ap()
    CT_d = nc.dram_tensor("CT", [128, S], fp32, kind="ExternalInput").ap()
    SST_d = nc.dram_tensor("SST", [128, S], fp32, kind="ExternalInput").ap()
    tri_d = nc.dram_tensor("tri", [128, 128], f16, kind="ExternalInput").ap()
    negI_d = nc.dram_tensor("negI", [128, 128], f16, kind="ExternalInput").ap()
    ones32_d = nc.dram_tensor("ones32", [128, 64], f32r, kind="ExternalInput").ap()
    # out packed as [4 chunks][8 ntiles][128, 512] contiguous blocks
    out_d = nc.dram_tensor("outp", [32, 128, 512], fp32, kind="ExternalOutput").ap()
    if debug:
        dbg_qT = nc.dram_tensor("dbg_qT", [2, 128, S], f16, kind="ExternalOutput").ap()
        dbg_kT = nc.dram_tensor("dbg_kT", [2, 128, S], f16, kind="ExternalOutput").ap()
        dbg_v = nc.dram_tensor("dbg_v", [128, 16 * 260], f16, kind="ExternalOutput").ap()
        dbg_zn = nc.dram_tensor("dbg_zn", [2, 128, S], f16, kind="ExternalOutput").ap()

    with tile.TileContext(nc) as tc, ExitStack() as ctx:
        const = ctx.enter_context(tc.tile_pool(name="const", bufs=1))
        qk_pool = ctx.enter_context(tc.tile_pool(name="qk", bufs=1))
        v_pool = ctx.enter_context(tc.tile_pool(name="v", bufs=1))
        zn_pool = ctx.enter_context(tc.tile_pool(name="zn", bufs=1))

        # SBUF tiles (declared before DMA issue order below)
        wq_big = const.tile([128, 9 * F], f16, tag="wqbig", name="wqbig")
        wk_big = const.tile([128, 9 * F], f16, tag="wkbig", name="wkbig")
        wv_big = const.tile([128, 9 * 260], f16, tag="wvbig", name="wvbig")
        xT_big = const.tile([128, 4 * 4608], f16, tag="xTbig", name="xTbig")
        CT_s = const.tile([128, S], fp32, tag="CT", name="CT")
        SST_s = const.tile([128, S], fp32, tag="SST", name="SST")
        mask_s = const.tile([128, 16], fp32, tag="maskv", name="maskv")
        tri_s = const.tile([128, 128], f16, tag="tri", name="tri")
        negI_s = const.tile([128, 128], f16, tag="negI", name="negI")
        ones32_s = const.tile([128, 64], f32r, tag="ones32", name="ones32")
        wo_big = const.tile([128, 2 * DIM], f16, tag="wobig", name="wobig")
        woT_s = [wo_big[:, DIM * t:DIM * t + DIM] for t in range(2)]
        wq_s = [wq_big[:, F * d:F * d + F] for d in range(9)]
        wk_s = [wk_big[:, F * d:F * d + F] for d in range(9)]
        wv_s = [wv_big[:, 260 * d:260 * d + 260] for d in range(9)]

        # xT chunk-major: chunk j occupies columns [4608j, 4608(j+1))
        def xs(d, j):
            return xT_big[:, 4608 * j + 512 * d:4608 * j + 512 * d + 512]

        def xtok(d, t):
            # 128-token-tile t of chunk t//4
            o = 4608 * (t // 4) + 512 * d + 128 * (t % 4)
            return xT_big[:, o:o + 128]

        # DMA issue order tuned for earliest first matmul; bulk loads are
        # single contiguous-per-partition transfers split across SP and ACT
        # descriptor generators.
        nc.sync.dma_start(wq_big[:], wqT_d[:])
        nc.sync.dma_start(xT_big[:, 0:4608], xT_d[0])
        nc.scalar.dma_start(CT_s[:], CT_d[:])
        nc.scalar.dma_start(SST_s[:], SST_d[:])
        nc.sync.dma_start(wk_big[:], wkT_d[:])
        nc.sync.dma_start(wv_big[:], wvTe_d[:])
        nc.scalar.dma_start(mask_s[:], mask_d.rearrange("(t p) one -> p (t one)", p=128))
        nc.scalar.dma_start(tri_s[:], tri_d[:])
        nc.scalar.dma_start(negI_s[:], negI_d[:])
        nc.scalar.dma_start(ones32_s[:], ones32_d[:])
        nc.sync.dma_start(xT_big[:, 4608:2 * 4608], xT_d[1])
        nc.scalar.dma_start(wo_big[:], woT_d[:])
        nc.sync.dma_start(xT_big[:, 2 * 4608:3 * 4608], xT_d[2])
        nc.sync.dma_start(xT_big[:, 3 * 4608:4 * 4608], xT_d[3])

        qhatT = [qk_pool.tile([128, S], f16, tag=f"qhatT{t}", name=f"qhatT{t}")
                 for t in range(2)]
        khatT = [qk_pool.tile([128, S], f16, tag=f"khatT{t}", name=f"khatT{t}")
                 for t in range(2)]
        v_big = v_pool.tile([128, 16 * 260], f16, tag="vbig", name="vbig")
        v_sb = [v_big[:, 260 * t:260 * t + 260] for t in range(16)]
        znT = [zn_pool.tile([128, S], f16, tag=f"znT{t}", name=f"znT{t}")
               for t in range(2)]

        # unified psum pool: 4 x [128,1024] = all 8 banks
        psp = ctx.enter_context(tc.tile_pool(name="psp", bufs=4, space="PSUM"))
        rope_p = ctx.enter_context(tc.tile_pool(name="ropet", bufs=3))
        pb_p = ctx.enter_context(tc.tile_pool(name="pb", bufs=1))
        zr_p = ctx.enter_context(tc.tile_pool(name="zr", bufs=3))
        rr_p = ctx.enter_context(tc.tile_pool(name="rr", bufs=4))
        ob_p = ctx.enter_context(tc.tile_pool(name="ob", bufs=4))

        pb_big = pb_p.tile([128, 24 * 1024], f16, tag="pbbig", name="pbbig")

        def out_tile(j, n):
            cs = slice(512 * j, 512 * j + 512)
            ps = psp.tile([128, 1024], fp32, tag="ps", name="ps")
            for t in range(2):
                MM(ps[:, 0:512], woT_s[t][:, 128 * n:128 * n + 128],
                   znT[t][:, cs], start=(t == 0), stop=(t == 1))
            ot = ob_p.tile([128, 512], fp32, tag="osb", name="osb")
            nc.vector.tensor_copy(ot[:], ps[:, 0:512])
            nc.sync.dma_start(out_d[8 * j + n], ot[:])

        def qk_group(j, which, fc):
            cs = slice(512 * j, 512 * j + 512)
            w_s, dst = (wq_s, qhatT) if which == "q" else (wk_s, khatT)
            ps = psp.tile([128, 1024], fp32, tag="ps", name="ps")
            for d in range(9):
                MM(ps[:, 0:512], w_s[d][:, 128 * fc:128 * fc + 128],
                   xs(d, j), start=(d == 0), stop=(d == 8))
            sw = rope_p.tile([128, 512], fp32, tag="sw", name="sw")
            nc.vector.stream_shuffle(sw[:], ps[:, 0:512], _SHUF_MASK)
            t1 = rope_p.tile([128, 512], fp32, tag="t1", name="t1")
            nc.gpsimd.tensor_mul(t1[:], sw[:], SST_s[:, cs])
            t2 = rope_p.tile([128, 512], fp32, tag="t2", name="t2")
            nc.vector.tensor_mul(t2[:], ps[:, 0:512], CT_s[:, cs])
            nc.gpsimd.tensor_add(dst[fc][:, cs], t1[:], t2[:])

        def v_group(j, t):
            ps = psp.tile([128, 1024], fp32, tag="ps", name="ps")
            for d in range(9):
                MM(ps[:, 0:260], xtok(d, t),
                   wv_s[d][:], start=(d == 0), stop=(d == 8))
            nc.vector.tensor_scalar_mul(v_sb[t][:], ps[:, 0:260],
                                        mask_s[:, t:t + 1])

        def z_head(h, j, pts_h):
            """z matmuls + eager psum evac for head h, chunk j."""
            ni = 4 * j + 4
            zps = psp.tile([128, 1024], fp32, tag="ps", name="ps")
            for i in range(ni):
                d = i - 4 * j
                c0 = max(0, 128 * d)
                MM(zps[0:65, c0:512], v_sb[i][:, 65 * h:65 * h + 65],
                   pts_h[i][:, c0:512], start=(i == 0), stop=(i == ni - 1))
            zraw = zr_p.tile([65, 512], f32r, tag="zraw", name="zraw")
            nc.vector.tensor_copy(zraw[:], zps[0:65, 0:512])
            return (h, j, zraw)

        def norm_head(st):
            """PE-broadcast the softmax sums into psum, reciprocal on DVE,
            then normalize into znT."""
            h, j, zraw = st
            tH, rH = h // 2, 64 * (h % 2)
            cs = slice(512 * j, 512 * j + 512)
            rps = psp.tile([128, 1024], fp32, tag="ps", name="ps")
            MM(rps[0:64, 0:512], ones32_s[64:65, :], zraw[64:65, :],
               start=True, stop=True)
            rinv = rr_p.tile([64, 512], fp32, tag="rinv", name="rinv")
            nc.vector.reciprocal_approx_fast(rinv[:], rps[0:64, 0:512])
            if rH == 0:
                nc.vector.tensor_mul(znT[tH][0:64, cs], zraw[0:64, :], rinv[:])
            else:
                tmp = rr_p.tile([64, 512], fp32, tag="ztmp", name="ztmp")
                nc.vector.tensor_mul(tmp[:], zraw[0:64, :], rinv[:])
                nc.vector.tensor_copy(znT[tH][64:128, cs], tmp[:])

        def scores_head(h, j, seam_cb=None):
            """scores + exp for head h, chunk j; returns p-tile map.
            seam_cb, if given, is invoked between score pairs to emit
            independent PE work that smooths the PE/ACT pipeline."""
            ni = 4 * j + 4
            tH, rH = h // 2, 64 * (h % 2)
            pts = {}
            for ip in range((ni + 1) // 2):
                sps = psp.tile([128, 1024], fp32, tag="ps", name="ps")
                slot = 8 * (h % (zdelay + 1)) + ip
                pb = pb_big[:, 1024 * slot:1024 * slot + 1024]
                cmin = 1024
                masks = []
                for ii in range(2):
                    i = 2 * ip + ii
                    if i >= ni:
                        continue
                    d = i - 4 * j
                    c0 = max(0, 128 * d)
                    cmin = min(cmin, 512 * ii + c0)
                    if d >= 0:
                        masks.append(512 * ii + 128 * d)
                    MM(sps[:, 512 * ii + c0:512 * ii + 512],
                       khatT[tH][rH:rH + 64, 128 * i:128 * i + 128],
                       qhatT[tH][rH:rH + 64, 512 * j + c0:512 * j + 512],
                       start=True, stop=(d < 0))
                    pts[i] = pb[:, 512 * ii:512 * ii + 512]
                for cc in masks:
                    # causal mask on PE: accumulate (-60000*I)^T @ tril01 so
                    # exp underflows to exactly 0 above the diagonal
                    MM(sps[:, cc:cc + 128], negI_s[:], tri_s[:],
                       start=False, stop=True)
                hi = 1024 if 2 * ip + 1 < ni else 512
                nc.scalar.activation(pb[:, cmin:hi], sps[:, cmin:hi], AF.Exp)
                if seam_cb is not None and ip % 2 == 1:
                    seam_cb()
            return pts

        # ---- prologue: projections for chunk 0 ----
        for fc in range(2):
            qk_group(0, "q", fc)
        for fc in range(2):
            qk_group(0, "k", fc)
        for t in range(4):
            v_group(0, t)

        pending = [None]      # deferred h3 norm from the previous chunk
        for j in range(4):
            # seam work: out_proj(j-1) tiles + proj/v groups for chunk j+1,
            # interleaved between attention heads so the PE never starves
            # while ACT drains exp.
            seams = []
            if j < 3:
                seams += [(qk_group, (j + 1, "q", 0)),
                          (qk_group, (j + 1, "k", 0)),
                          (qk_group, (j + 1, "q", 1)),
                          (qk_group, (j + 1, "k", 1))]
            # out tiles are deferred by TWO chunks: chunk 3 is ACT(exp)-bound
            # and otherwise has little independent seam work for the PE
            if j >= 2:
                seams += [(out_tile, (j - 2, n)) for n in range(8)]
            if j == 3:
                seams += [(out_tile, (2, n)) for n in range(8)]
            if j < 3:
                seams += [(v_group, (j + 1, t)) for t in range(4 * j + 4,
                                                               4 * j + 8)]
            if pending[0] is not None:
                st = pending[0]
                pending[0] = None
                seams = [(norm_head, (st,))] + seams
            seams = seams[::-1]          # pop() from the front of the list
            per_seam = max(1, (len(seams) + HPC - 1) // HPC)

            zdelay = 1
            hq = []              # (h, pts) awaiting z
            zq = []
            def seam_cb():
                if seams:
                    fn, args = seams.pop()
                    fn(*args)

            # head order alternates parity and ends on an even head so the
            # tail-critical final normalize writes znT directly (no copy)
            for h in (1, 0, 3, 2):
                hq.append((h, scores_head(h, j, seam_cb if j >= 1 else None)))
                if j < 2:
                    for _ in range(per_seam):
                        seam_cb()
                if len(hq) > zdelay:
                    ph, ppts = hq.pop(0)
                    zq.append(z_head(ph, j, ppts))
                if len(zq) >= 2:
                    norm_head(zq.pop(0))
            while hq:
                ph, ppts = hq.pop(0)
                zq.append(z_head(ph, j, ppts))
            while seams:
                fn, args = seams.pop()
                fn(*args)
            while len(zq) > 1:
                norm_head(zq.pop(0))
            pending[0] = zq.pop(0)

        norm_head(pending[0])
        out_proj3 = [out_tile(3, n) for n in range(8)]

        if debug:
            for t in range(2):
                nc.sync.dma_start(dbg_qT[t], qhatT[t][:])
                nc.sync.dma_start(dbg_kT[t], khatT[t][:])
                nc.sync.dma_start(dbg_zn[t], znT[t][:])
            nc.sync.dma_start(dbg_v[:], v_big[:])

    nc.finalize()
    return nc


_NC_CACHE = {}


def kernel(x, attn_mask, wq_w, wq_b, wk_w, wk_b, wv_w, wv_b, wo_w, wo_b):
    from concourse.bass_utils import run_bass_kernel_spmd

    x = np.asarray(x, np.float32)
    attn_mask = np.asarray(attn_mask)
    wq_w = np.asarray(wq_w, np.float32); wq_b = np.asarray(wq_b, np.float32)
    wk_w = np.asarray(wk_w, np.float32); wk_b = np.asarray(wk_b, np.float32)
    wv_w = np.asarray(wv_w, np.float32); wv_b = np.asarray(wv_b, np.float32)
    wo_w = np.asarray(wo_w, np.float32); wo_b = np.asarray(wo_b, np.float32)

    CT, SST = _rope_tables()
    tri01 = np.tril(np.ones((128, 128), np.float16), -1)

    hp = _head_perm()
    perm = np.concatenate([64 * h + hp for h in range(HPC)])  # [256]

    in_maps = []
    for c in range(NC):
        b, g = c // 4, c % 4
        fs = slice(F * g, F * g + F)
        wq = wq_w[fs][perm] / np.float32(8.0)
        wk = wk_w[fs][perm]
        qb = wq_b[fs][perm] / np.float32(8.0)
        kb = wk_b[fs][perm]
        wqTe = np.zeros((DIM + 128, F), np.float16)
        wqTe[0:DIM] = wq.T.astype(np.float16)
        wqTe[DIM] = qb.astype(np.float16)
        wkTe = np.zeros((DIM + 128, F), np.float16)
        wkTe[0:DIM] = wk.T.astype(np.float16)
        wkTe[DIM] = kb.astype(np.float16)
        wv = wv_w[fs]
        vb = wv_b[fs]
        wvTe = np.zeros((DIM + 128, 260), np.float16)
        for h in range(HPC):
            wvTe[0:DIM, 65 * h:65 * h + 64] = wv[64 * h:64 * h + 64].T.astype(np.float16)
            wvTe[DIM, 65 * h:65 * h + 64] = vb[64 * h:64 * h + 64].astype(np.float16)
            wvTe[DIM, 65 * h + 64] = 1.0
        xTe = np.zeros((DIM + 128, S), np.float16)
        xTe[0:DIM] = x[b].T.astype(np.float16)
        xTe[DIM] = 1.0
        # pack xT chunk-major: [4 chunks][128 partitions][9 dtiles * 512]
        xTb = np.ascontiguousarray(
            xTe.reshape(9, 128, 4, 512).transpose(2, 1, 0, 3)
        ).reshape(4, 128, 9 * 512)
        woTc = np.ascontiguousarray(wo_w[:, fs].T).astype(np.float16)
        in_maps.append({
            "xT": xTb,
            "wqT": np.ascontiguousarray(
                wqTe.reshape(9, 128, F).transpose(1, 0, 2)).reshape(128, 9 * F),
            "wkT": np.ascontiguousarray(
                wkTe.reshape(9, 128, F).transpose(1, 0, 2)).reshape(128, 9 * F),
            "wvTe": np.ascontiguousarray(
                wvTe.reshape(9, 128, 260).transpose(1, 0, 2)).reshape(128, 9 * 260),
            "woT": np.ascontiguousarray(
                woTc.reshape(2, 128, DIM).transpose(1, 0, 2)).reshape(128, 2 * DIM),
            "maskv": attn_mask[b].astype(np.float32).reshape(S, 1).copy(),
            "CT": CT, "SST": SST, "tri": tri01,
            "negI": (np.eye(128) * -60000.0).astype(np.float16),
            "ones32": np.ones((128, 64), np.float32),
        })

    if "nc" not in _NC_CACHE:
        _NC_CACHE["nc"] = _build_program()
    res = run_bass_kernel_spmd(_NC_CACHE["nc"], in_maps, core_ids=list(range(NC)))
    globals()["LAST_RESULTS"] = res

    out = np.zeros((B, DIM, S), np.float32)
    for c in range(NC):
        # outp [32,128,512] = [4 chunks][8 ntiles][128,512] -> [1024, 2048]
        o = res.results[c]["outp"].reshape(4, 8, 128, 512)
        out[c // 4] += o.transpose(1, 2, 0, 3).reshape(DIM, S)
    out = out.transpose(0, 2, 1) + wo_b[None, None, :]
    return np.ascontiguousarray(out)


if __name__ == "__main__":
    rng = np.random.default_rng(0)
    ins = {
        "x": rng.standard_normal((B, S, DIM)).astype(np.float32),
        "attn_mask": np.ones((B, S), bool),
    }
    for n in ["wq", "wk", "wv", "wo"]:
        ins[n + "_w"] = (rng.standard_normal((DIM, DIM)).astype(np.float32) / 32.0)
        ins[n + "_b"] = rng.standard_normal(DIM).astype(np.float32) * 0.01
    o = kernel(**ins)
    print("ran", o.shape, o.dtype)
